# revision 1
# baseline (speedup 1.0000x reference)
"""Trainium2 Bass kernel for CausalGraphLayer (gnn message passing).

out[b,n,t,c] = tanh( sum_k w[n,k,c] * z[b, idx[n,k], t, c] )
  w[n,k,c] = adjacency[n,k] * sum_bb channel_coeffs[c,bb] * basis_weights[bb,n,k]

Decomposition used on device (per core, nodes sharded 8 ways):
  G[bb,n,f]   = sum_k (adj*basis)[bb,n,k] * z_cat[idx[n,k], f]   (PE, k-contraction)
  out[n,f]    = tanh( sum_bb coeffs[c(f),bb] * G[bb,n,f] )       (DVE mask + PE quad-reduce + ACT)
with f = (b, t, c) fused so each gathered row is 8KB (both batches).
"""

import sys

if "/opt/trn_rl_repo" not in sys.path:
    sys.path.insert(0, "/opt/trn_rl_repo")

import numpy as np

import concourse.bass as bass
import concourse.tile as tile
from concourse import bacc, mybir
from concourse.bass_utils import run_bass_kernel_spmd

# Problem constants (nn_CausalGraphLayer_22050362098277)
B, N, T, C = 2, 2048, 32, 32
NUM_BASES, K_CURR = 4, 16
N_CORES = 8
N_LOC = N // N_CORES            # 256 nodes per core
GROUP = 8                       # nodes per gather tile (8 nodes x 16 neigh = 128 slots)
N_GROUPS = N_LOC // GROUP       # 32 groups per core
STACK = 4                       # groups per 128-partition stack
N_STACKS = N_GROUPS // STACK    # 8 stacks per core
F = B * T * C                   # 2048 fused free dim
FCH = 512                       # PSUM bank chunk (fp32)
N_CH = F // FCH                 # 4 chunks

_compiled_cache = {}


def _build_program(reps: int = 1):
    """Build the SPMD Bass program (identical on all cores; per-core data
    differences are carried in the input tensors)."""
    nc = bacc.Bacc(
        "TRN2", target_bir_lowering=False, debug=False, num_devices=N_CORES
    )
    f32 = mybir.dt.float32
    z_cat = nc.dram_tensor("z_cat", [N, F], f32, kind="ExternalInput").ap()
    idx_d = nc.dram_tensor("idx", [128, N_GROUPS], mybir.dt.int32,
                           kind="ExternalInput").ap()
    w_d = nc.dram_tensor("w", [128, N_GROUPS * 32], f32, kind="ExternalInput").ap()
    cm_d = nc.dram_tensor("cm", [128, F], f32, kind="ExternalInput").ap()
    sel_d = nc.dram_tensor("sel", [128, 32], f32, kind="ExternalInput").ap()
    out_d = nc.dram_tensor("out", [N_LOC, F], f32, kind="ExternalOutput").ap()

    with tile.TileContext(nc) as tc:
        with (
            tc.tile_pool(name="const", bufs=1) as cpool,
            tc.tile_pool(name="rbuf", bufs=8) as rpool,
            tc.tile_pool(name="tmp", bufs=4) as tpool,
            tc.tile_pool(name="obuf", bufs=2) as opool,
            tc.tile_pool(name="gps", bufs=4, space="PSUM") as gpool,
            tc.tile_pool(name="ops", bufs=2, space="PSUM") as o3pool,
        ):
            cm_sb = cpool.tile([128, F], f32)
            nc.sync.dma_start(cm_sb[:], cm_d[:, :])
            sel_sb = cpool.tile([128, 32], f32)
            nc.sync.dma_start(sel_sb[:], sel_d[:, :])
            w_sb = cpool.tile([128, N_GROUPS * 32], f32)
            nc.sync.dma_start(w_sb[:], w_d[:, :])
            idx_sb = cpool.tile([128, N_GROUPS], mybir.dt.int32)
            nc.sync.dma_start(idx_sb[:], idx_d[:, :])

            for _rep in range(reps):
                for blk in range(N_STACKS // STACK):       # 2 blocks of 128 nodes
                    outbuf = opool.tile([128, F], f32)
                    for sq in range(STACK):                # 4 stacks of 32 nodes
                        s = STACK * blk + sq
                        rts = []
                        for gq in range(STACK):
                            g = STACK * s + gq
                            r = rpool.tile([128, F], f32)
                            nc.gpsimd.indirect_dma_start(
                                out=r[:],
                                out_offset=None,
                                in_=z_cat[:],
                                in_offset=bass.IndirectOffsetOnAxis(
                                    ap=idx_sb[:, g:g + 1], axis=0
                                ),
                            )
                            rts.append(r)
                        for ch in range(N_CH):
                            gc = gpool.tile([128, FCH], f32)
                            for gq in range(STACK):
                                g = STACK * s + gq
                                nc.tensor.matmul(
                                    out=gc[32 * gq:32 * (gq + 1), :],
                                    lhsT=w_sb[:, 32 * g:32 * (g + 1)],
                                    rhs=rts[gq][:, FCH * ch:FCH * (ch + 1)],
                                    start=True, stop=True,
                                    tile_position=(0, 32 * gq),
                                )
                            tmpc = tpool.tile([128, FCH], f32)
                            nc.vector.tensor_tensor(
                                out=tmpc[:], in0=gc[:],
                                in1=cm_sb[:, FCH * ch:FCH * (ch + 1)],
                                op=mybir.AluOpType.mult,
                            )
                            o3 = o3pool.tile([32, FCH], f32)
                            nc.tensor.matmul(
                                out=o3[:], lhsT=sel_sb[:], rhs=tmpc[:],
                                start=True, stop=True, tile_position=(0, 0),
                            )
                            nc.scalar.activation(
                                out=outbuf[32 * sq:32 * (sq + 1),
                                           FCH * ch:FCH * (ch + 1)],
                                in_=o3[:],
                                func=mybir.ActivationFunctionType.Tanh,
                            )
                    nc.sync.dma_start(
                        out_d[128 * blk:128 * (blk + 1), :], outbuf[:]
                    )

    nc.compile()
    return nc


def _prep_inputs(z, neighbor_indices, adjacency, basis_weights, channel_coeffs):
    """Host-side packing of inputs into the per-core device tensors."""
    z = np.asarray(z, dtype=np.float32)
    nbr = np.asarray(neighbor_indices).astype(np.int32)      # [N, 16]
    adj = np.asarray(adjacency, dtype=np.float32)[:, :K_CURR]
    basis = np.asarray(basis_weights, dtype=np.float32)[:, :, :K_CURR]
    coeffs = np.asarray(channel_coeffs, dtype=np.float32)    # [C, NUM_BASES]

    # z_cat[m, b*T*C + t*C + c] = z[b, m, t, c]
    z_cat = np.ascontiguousarray(
        z.transpose(1, 0, 2, 3).reshape(N, F)
    )

    # wb[bb, n, k] = adj[n,k] * basis[bb,n,k]
    wb = adj[None, :, :] * basis                              # [4, N, 16]

    # coeff mask CM[p, f] = coeffs[c(f), p % 4]
    c_of_f = np.tile(np.arange(C), B * T)                     # [F]
    CM = np.ascontiguousarray(
        coeffs[c_of_f[None, :], (np.arange(128) % NUM_BASES)[:, None]]
    ).astype(np.float32)                                      # [128, F]

    # SEL[p, m] = 1 iff p == 32*(m//8) + 4*(m%8) + bb  for some bb
    p = np.arange(128)
    m = np.arange(32)
    SEL = ((p[:, None] // 32 == m[None, :] // 8)
           & ((p[:, None] % 32) // 4 == m[None, :] % 8)).astype(np.float32)

    in_maps = []
    ii = np.arange(GROUP)
    for r in range(N_CORES):
        lo = r * N_LOC
        nbr_c = nbr[lo:lo + N_LOC]                            # [256, 16]
        idx_core = nbr_c.reshape(N_GROUPS, GROUP * K_CURR)    # [32, 128]
        idx_dram = np.ascontiguousarray(idx_core.T)           # [128, 32]

        wb_c = wb[:, lo:lo + N_LOC, :]                        # [4, 256, 16]
        w_g = wb_c.reshape(NUM_BASES, N_GROUPS, GROUP, K_CURR)
        w_g = w_g.transpose(1, 2, 3, 0)                       # [g, i, k, bb]
        W5 = np.zeros((N_GROUPS, GROUP, K_CURR, GROUP, NUM_BASES),
                      dtype=np.float32)
        W5[:, ii, :, ii, :] = w_g.transpose(1, 0, 2, 3)
        w_dram = np.ascontiguousarray(
            W5.reshape(N_GROUPS, 128, 32).transpose(1, 0, 2).reshape(128, -1)
        )

        in_maps.append({
            "z_cat": z_cat,
            "idx": idx_dram,
            "w": w_dram,
            "cm": CM,
            "sel": SEL,
        })
    return in_maps


def kernel(z, neighbor_indices, adjacency, basis_weights, channel_coeffs):
    assert z.shape == (B, N, T, C), z.shape
    key = "prog1"
    if key not in _compiled_cache:
        _compiled_cache[key] = _build_program(reps=1)
    nc = _compiled_cache[key]

    in_maps = _prep_inputs(z, neighbor_indices, adjacency,
                           basis_weights, channel_coeffs)
    res = run_bass_kernel_spmd(nc, in_maps, core_ids=list(range(N_CORES)))
    out_cat = np.stack([r["out"] for r in res.results])       # [8, 256, F]
    out = (out_cat.reshape(N, B, T, C).transpose(1, 0, 2, 3))
    return np.ascontiguousarray(out)

